# revision 11
# baseline (speedup 1.0000x reference)
"""Bidirectional Mamba on 8 Trainium2 NeuronCores.

Sharding: 8 cores = (2 directions) x (4 batch elements); each core runs one
full Mamba block on its (L=1024, DM=512) sequence. The backward direction is
handled by flipping the sequence on the host before/after, so all cores run
the identical SPMD program with different data.

Per-core layout: channels d on partitions (4 chunks of 128), time t on the
free dim. The selective scan runs as one tensor_tensor_scan per (d-chunk, n)
with state in fp32. B/C rows are broadcast across partitions with GPSIMD
partition_broadcast; all GEMMs keep contraction on the partition axis so no
on-device transposes are needed anywhere.
"""
import numpy as np

import concourse.bacc as bacc
import concourse.tile as tile
import concourse.mybir as mybir
from concourse.bass_utils import run_bass_kernel_spmd

F32 = mybir.dt.float32
AF = mybir.ActivationFunctionType
OP = mybir.AluOpType

DM = 512
DI = 512
L = 1024
N = 16
K = 4
R = 32
P = 128
NCH = DI // P          # 4 d-chunks
TB = 512               # t-block for matmul moving operand
NTB = L // TB          # 2
N_CORES = 8


def emit_mamba(ctx_stack, tc, io):
    import contextlib

    nc = tc.nc
    f32 = F32

    with contextlib.ExitStack() as ctx:
        # ---- persistent SBUF tiles ----
        per = ctx.enter_context(tc.tile_pool(name="per", bufs=1))

        def ptile(tag, shape, dtype=f32):
            return per.tile(shape, dtype, tag=tag, name=tag)

        Wc_sb = [ptile(f"Wc{i}", [P, K]) for i in range(NCH)]
        bconv_sb = [ptile(f"bcv{i}", [P, 1]) for i in range(NCH)]
        Wx_sb = [ptile(f"Wx{i}", [P, R + 2 * N]) for i in range(NCH)]
        Wdt_sb = ptile("Wdt", [R, DI])
        bdt_sb = [ptile(f"bdt{i}", [P, 1]) for i in range(NCH)]
        A_sb = [ptile(f"A{i}", [P, N]) for i in range(NCH)]
        D_sb = [ptile(f"D{i}", [P, 1]) for i in range(NCH)]
        Wout_sb = [ptile(f"Wo{i}", [P, DM]) for i in range(NCH)]
        ones_sb = ptile("ones", [1, P])

        xc_sb = [ptile(f"xc{i}", [P, L]) for i in range(NCH)]
        zs_sb = [ptile(f"zs{i}", [P, L]) for i in range(NCH)]
        xs_sb = [ptile(f"xs{i}", [P, L]) for i in range(NCH)]
        dt_sb = [ptile(f"dt{i}", [P, L]) for i in range(NCH)]
        u_sb = [ptile(f"u{i}", [P, L]) for i in range(NCH)]
        y_sb = [ptile(f"y{i}", [P, L]) for i in range(NCH)]
        yz_sb = [ptile(f"yz{i}", [P, L]) for i in range(NCH)]
        dbc_sb = ptile("dbc", [R + 2 * N, L])

        for i in range(NCH):
            sl = slice(i * P, (i + 1) * P)
            nc.sync.dma_start(Wc_sb[i][:], io["Wc"][sl, :])
            nc.sync.dma_start(bconv_sb[i][:], io["bconv"][sl, :])
            nc.sync.dma_start(Wx_sb[i][:], io["Wx"][sl, :])
            nc.sync.dma_start(bdt_sb[i][:], io["bdt"][sl, :])
            nc.sync.dma_start(A_sb[i][:], io["A_sc"][sl, :])
            nc.sync.dma_start(D_sb[i][:], io["Dv"][sl, :])
            nc.sync.dma_start(Wout_sb[i][:], io["W_out"][sl, :])
        nc.sync.dma_start(Wdt_sb[:], io["Wdt"][:, :])
        nc.sync.dma_start(ones_sb[:], io["ones"][:, :])

        # ---- GEMM A: xz_T = W_in^T @ x_T ; silu on z half ----
        with tc.tile_pool(name="gin", bufs=1) as gin, tc.tile_pool(
            name="psA", bufs=4, space="PSUM"
        ) as psA:
            W_in_sb = [
                gin.tile([P, 2 * DI], f32, tag=f"Wi{i}", name=f"Wi{i}")
                for i in range(NCH)
            ]
            xT_sb = [
                gin.tile([P, L], f32, tag=f"xT{i}", name=f"xT{i}")
                for i in range(NCH)
            ]
            for i in range(NCH):
                sl = slice(i * P, (i + 1) * P)
                nc.sync.dma_start(W_in_sb[i][:], io["W_in"][sl, :])
                nc.sync.dma_start(xT_sb[i][:], io["xT"][sl, :])

            for cb in range(2 * DI // P):  # 8 output blocks of 128 channels
                for tb in range(NTB):
                    ps = psA.tile([P, TB], f32, tag="psA")
                    for mk in range(NCH):
                        nc.tensor.matmul(
                            ps[:],
                            lhsT=W_in_sb[mk][:, cb * P : (cb + 1) * P],
                            rhs=xT_sb[mk][:, tb * TB : (tb + 1) * TB],
                            start=(mk == 0),
                            stop=(mk == NCH - 1),
                        )
                    tsl = slice(tb * TB, (tb + 1) * TB)
                    if cb < NCH:
                        nc.scalar.activation(xc_sb[cb][:, tsl], ps[:], AF.Copy)
                    else:
                        nc.scalar.activation(zs_sb[cb - NCH][:, tsl], ps[:], AF.Silu)

        # ---- causal depthwise conv (K=4) + silu -> xs ----
        with tc.tile_pool(name="cv", bufs=2) as cvp:
            for dc in range(NCH):
                xcv = cvp.tile([P, L], f32, tag="xcv")
                nc.vector.tensor_scalar_mul(xcv[:], xc_sb[dc][:], Wc_sb[dc][:, 3:4])
                for k in (2, 1, 0):
                    s = K - 1 - k
                    nc.vector.scalar_tensor_tensor(
                        out=xcv[:, s:],
                        in0=xc_sb[dc][:, : L - s],
                        scalar=Wc_sb[dc][:, k : k + 1],
                        in1=xcv[:, s:],
                        op0=OP.mult,
                        op1=OP.add,
                    )
                nc.scalar.activation(
                    xs_sb[dc][:], xcv[:], AF.Silu, bias=bconv_sb[dc][:, 0:1]
                )

        # ---- GEMM B: dbc_T = W_xproj^T @ xs_T  (64 rows: dt_in | B | C) ----
        with tc.tile_pool(name="psB", bufs=2, space="PSUM") as psB:
            for tb in range(NTB):
                ps = psB.tile([R + 2 * N, TB], f32, tag="psB")
                for dc in range(NCH):
                    nc.tensor.matmul(
                        ps[:],
                        lhsT=Wx_sb[dc][:],
                        rhs=xs_sb[dc][:, tb * TB : (tb + 1) * TB],
                        start=(dc == 0),
                        stop=(dc == NCH - 1),
                    )
                nc.scalar.activation(
                    dbc_sb[:, tb * TB : (tb + 1) * TB], ps[:], AF.Copy
                )

        # ---- GEMM C: dt_T = softplus(W_dt^T @ dt_in_T + b_dt) ----
        # softplus(x) = ln(1 + exp(x)); the ACT softplus table isn't available,
        # but exp and ln share one table set (natural_log_exp_and_others).
        with tc.tile_pool(name="psC", bufs=2, space="PSUM") as psC, tc.tile_pool(
            name="spl", bufs=2
        ) as spl:
            for dc in range(NCH):
                for tb in range(NTB):
                    ps = psC.tile([P, TB], f32, tag="psC")
                    nc.tensor.matmul(
                        ps[:],
                        lhsT=Wdt_sb[:, dc * P : (dc + 1) * P],
                        rhs=dbc_sb[0:R, tb * TB : (tb + 1) * TB],
                        start=True,
                        stop=True,
                    )
                    et = spl.tile([P, TB], f32, tag="et")
                    nc.scalar.activation(
                        et[:], ps[:], AF.Exp, bias=bdt_sb[dc][:, 0:1]
                    )
                    nc.scalar.activation(
                        dt_sb[dc][:, tb * TB : (tb + 1) * TB],
                        et[:],
                        AF.Ln,
                        bias=1.0,
                    )

        # u = dt * xs
        for dc in range(NCH):
            nc.vector.tensor_mul(u_sb[dc][:], dt_sb[dc][:], xs_sb[dc][:])

        # ---- selective scan: n outer (broadcasts shared), d-chunk inner ----
        # B/C rows are staged at partition 0 via small SBUF->SBUF DMAs
        # (compute-engine APs can't start at arbitrary partitions, DMA can
        # read them), then broadcast across partitions with K=1 PE matmuls.
        with tc.tile_pool(name="scan", bufs=3) as sp, tc.tile_pool(
            name="rowp", bufs=3
        ) as rowp, tc.tile_pool(
            name="psBb", bufs=2, space="PSUM"
        ) as psBb, tc.tile_pool(name="psCb", bufs=2, space="PSUM") as psCb:
            for n in range(N):
                Brow = rowp.tile([1, L], f32, tag="Brow")
                nc.sync.dma_start(Brow[:], dbc_sb[R + n : R + n + 1, :])
                Crow = rowp.tile([1, L], f32, tag="Crow")
                nc.sync.dma_start(Crow[:], dbc_sb[R + N + n : R + N + n + 1, :])
                Bb = psBb.tile([P, L], f32, tag="Bb")
                Cb = psCb.tile([P, L], f32, tag="Cb")
                for tb in range(NTB):
                    tsl = slice(tb * TB, (tb + 1) * TB)
                    nc.tensor.matmul(
                        Bb[:, tsl], lhsT=ones_sb[:], rhs=Brow[:, tsl],
                        start=True, stop=True,
                    )
                    nc.tensor.matmul(
                        Cb[:, tsl], lhsT=ones_sb[:], rhs=Crow[:, tsl],
                        start=True, stop=True,
                    )
                for dc in range(NCH):
                    dA = sp.tile([P, L], f32, tag="dA")
                    nc.scalar.activation(
                        dA[:], dt_sb[dc][:], AF.Exp, scale=A_sb[dc][:, n : n + 1]
                    )
                    dBx = sp.tile([P, L], f32, tag="dBx")
                    nc.vector.tensor_mul(dBx[:], u_sb[dc][:], Bb[:])
                    h = sp.tile([P, L], f32, tag="h")
                    nc.vector.tensor_tensor_scan(
                        h[:], dA[:], dBx[:], 0.0, op0=OP.mult, op1=OP.add
                    )
                    if n == 0:
                        nc.vector.tensor_mul(y_sb[dc][:], h[:], Cb[:])
                    else:
                        hC = sp.tile([P, L], f32, tag="hC")
                        nc.vector.tensor_mul(hC[:], h[:], Cb[:])
                        nc.vector.tensor_add(y_sb[dc][:], y_sb[dc][:], hC[:])

        # yz = (y + D*xs) * silu(z)
        for dc in range(NCH):
            nc.vector.scalar_tensor_tensor(
                out=yz_sb[dc][:],
                in0=xs_sb[dc][:],
                scalar=D_sb[dc][:, 0:1],
                in1=y_sb[dc][:],
                op0=OP.mult,
                op1=OP.add,
            )
            nc.vector.tensor_mul(yz_sb[dc][:], yz_sb[dc][:], zs_sb[dc][:])

        # ---- GEMM D: out_T = W_out^T @ yz_T ----
        with tc.tile_pool(name="psD", bufs=4, space="PSUM") as psD, tc.tile_pool(
            name="osb", bufs=4
        ) as osb:
            for mb in range(DM // P):
                for tb in range(NTB):
                    ps = psD.tile([P, TB], f32, tag="psD")
                    for dc in range(NCH):
                        nc.tensor.matmul(
                            ps[:],
                            lhsT=Wout_sb[dc][:, mb * P : (mb + 1) * P],
                            rhs=yz_sb[dc][:, tb * TB : (tb + 1) * TB],
                            start=(dc == 0),
                            stop=(dc == NCH - 1),
                        )
                    ot = osb.tile([P, TB], f32, tag="ot")
                    nc.scalar.activation(ot[:], ps[:], AF.Copy)
                    nc.sync.dma_start(
                        io["outT"][mb * P : (mb + 1) * P, tb * TB : (tb + 1) * TB],
                        ot[:],
                    )


def build(reps=1):
    nc = bacc.Bacc(
        "TRN2",
        target_bir_lowering=False,
        debug=False,
        enable_asserts=False,
        num_devices=N_CORES,
    )
    io = {
        "xT": nc.dram_tensor("xT", (DM, L), F32, kind="ExternalInput").ap(),
        "W_in": nc.dram_tensor("W_in", (DM, 2 * DI), F32, kind="ExternalInput").ap(),
        "Wc": nc.dram_tensor("Wc", (DI, K), F32, kind="ExternalInput").ap(),
        "bconv": nc.dram_tensor("bconv", (DI, 1), F32, kind="ExternalInput").ap(),
        "Wx": nc.dram_tensor("Wx", (DI, R + 2 * N), F32, kind="ExternalInput").ap(),
        "Wdt": nc.dram_tensor("Wdt", (R, DI), F32, kind="ExternalInput").ap(),
        "bdt": nc.dram_tensor("bdt", (DI, 1), F32, kind="ExternalInput").ap(),
        "A_sc": nc.dram_tensor("A_sc", (DI, N), F32, kind="ExternalInput").ap(),
        "Dv": nc.dram_tensor("Dv", (DI, 1), F32, kind="ExternalInput").ap(),
        "W_out": nc.dram_tensor("W_out", (DI, DM), F32, kind="ExternalInput").ap(),
        "ones": nc.dram_tensor("ones", (1, P), F32, kind="ExternalInput").ap(),
        "outT": nc.dram_tensor("outT", (DM, L), F32, kind="ExternalOutput").ap(),
    }
    with tile.TileContext(nc) as tc:
        for _ in range(reps):
            emit_mamba(None, tc, io)
    nc.compile()
    return nc


_NC_CACHE = {}


def _get_nc(reps=1):
    if reps not in _NC_CACHE:
        _NC_CACHE[reps] = build(reps)
    return _NC_CACHE[reps]


def make_in_maps(inputs):
    x = np.asarray(inputs["x"], np.float32)
    in_maps = []
    for c in range(N_CORES):
        b = c % 4
        sfx = "f" if c < 4 else "b"
        xb = x[b] if c < 4 else x[b][::-1]

        def g(name):
            return np.asarray(inputs[f"{name}_{sfx}"], np.float32)

        in_maps.append(
            {
                "xT": np.ascontiguousarray(xb.T),
                "W_in": np.ascontiguousarray(g("W_in")),
                "Wc": np.ascontiguousarray(g("W_conv")),
                "bconv": np.ascontiguousarray(g("b_conv").reshape(DI, 1)),
                "Wx": np.ascontiguousarray(g("W_xproj")),
                "Wdt": np.ascontiguousarray(g("W_dt")),
                "bdt": np.ascontiguousarray(g("b_dt").reshape(DI, 1)),
                "A_sc": np.ascontiguousarray(-np.exp(g("A_log"))),
                "Dv": np.ascontiguousarray(g("D").reshape(DI, 1)),
                "W_out": np.ascontiguousarray(g("W_out")),
                "ones": np.ones((1, P), np.float32),
            }
        )
    return in_maps


def assemble_output(results):
    out = np.empty((4, L, DM), np.float32)
    for b in range(4):
        of = results[b]["outT"].T
        ob = results[4 + b]["outT"].T[::-1]
        out[b] = of + ob
    return out


def kernel(**inputs):
    nc = _get_nc()
    in_maps = make_in_maps(inputs)
    res = run_bass_kernel_spmd(nc, in_maps, core_ids=list(range(N_CORES)))
    return assemble_output(res.results)


# revision 14
# speedup vs baseline: 1.1820x; 1.1820x over previous
"""Bidirectional Mamba on 8 Trainium2 NeuronCores.

Sharding: 8 cores = (2 directions) x (4 batch elements); each core runs one
full Mamba block on its (L=1024, DM=512) sequence. The backward direction is
handled by flipping the sequence on the host before/after, so all cores run
the identical SPMD program with different data.

Per-core layout: channels d on partitions (4 chunks of 128), time t on the
free dim. The selective scan runs as one tensor_tensor_scan per (d-chunk, n)
with fp32 state. B/C rows are staged at partition 0 by DMA, broadcast across
partitions with K=1 PE matmuls, and y = sum_n h_n*C_n accumulates in PSUM via
identity matmuls on the PE so the vector engine only carries the scan and the
dBx multiply. GEMMs run as float32r (full-rate PE path).
"""
import contextlib

import numpy as np

import concourse.bacc as bacc
import concourse.tile as tile
import concourse.mybir as mybir
from concourse.bass_utils import run_bass_kernel_spmd

F32 = mybir.dt.float32
F32R = mybir.dt.float32r
AF = mybir.ActivationFunctionType
OP = mybir.AluOpType

DM = 512
DI = 512
L = 1024
N = 16
K = 4
R = 32
P = 128
NCH = DI // P          # 4 d-chunks
TB = 512               # t-block for matmul moving operand
NTB = L // TB          # 2
N_CORES = 8

USE_F32R_GEMM = False  # plain fp32 GEMMs (exact; PE has headroom)
USE_F32R_AUX = True    # broadcasts + y-accumulate in float32r


def _mm(nc, out, lhsT, rhs, start, stop, f32r):
    if f32r:
        lhsT = lhsT.bitcast(F32R)
        rhs = rhs.bitcast(F32R)
    nc.tensor.matmul(out, lhsT=lhsT, rhs=rhs, start=start, stop=stop,
                     skip_group_check=True)


def emit_mamba(tc, io):
    nc = tc.nc
    f32 = F32

    with contextlib.ExitStack() as ctx:
        # ---- persistent SBUF tiles ----
        per = ctx.enter_context(tc.tile_pool(name="per", bufs=1))

        def ptile(tag, shape, dtype=f32):
            return per.tile(shape, dtype, tag=tag, name=tag)

        Wc_sb = [ptile(f"Wc{i}", [P, K]) for i in range(NCH)]
        bconv_sb = [ptile(f"bcv{i}", [P, 1]) for i in range(NCH)]
        Wx_sb = [ptile(f"Wx{i}", [P, R + 2 * N]) for i in range(NCH)]
        Wdt_sb = ptile("Wdt", [R, DI])
        bdt_sb = [ptile(f"bdt{i}", [P, 1]) for i in range(NCH)]
        A_sb = [ptile(f"A{i}", [P, N]) for i in range(NCH)]
        D_sb = [ptile(f"D{i}", [P, 1]) for i in range(NCH)]
        Wout_sb = [ptile(f"Wo{i}", [P, DM]) for i in range(NCH)]
        ones_sb = ptile("ones", [1, P], F32R)
        ident_sb = ptile("ident", [P, P], F32R)

        xc_sb = [ptile(f"xc{i}", [P, L]) for i in range(NCH)]
        zs_sb = [ptile(f"zs{i}", [P, L]) for i in range(NCH)]
        xs_sb = [ptile(f"xs{i}", [P, L]) for i in range(NCH)]
        dt_sb = [ptile(f"dt{i}", [P, L]) for i in range(NCH)]
        u_sb = [ptile(f"u{i}", [P, L]) for i in range(NCH)]
        yz_sb = [ptile(f"yz{i}", [P, L]) for i in range(NCH)]
        dbc_sb = ptile("dbc", [R + 2 * N, L])

        for i in range(NCH):
            sl = slice(i * P, (i + 1) * P)
            nc.sync.dma_start(Wc_sb[i][:], io["Wc"][sl, :])
            nc.sync.dma_start(bconv_sb[i][:], io["bconv"][sl, :])
            nc.sync.dma_start(Wx_sb[i][:], io["Wx"][sl, :])
            nc.sync.dma_start(bdt_sb[i][:], io["bdt"][sl, :])
            nc.sync.dma_start(A_sb[i][:], io["A_sc"][sl, :])
            nc.sync.dma_start(D_sb[i][:], io["Dv"][sl, :])
            nc.sync.dma_start(Wout_sb[i][:], io["W_out"][sl, :])
        nc.sync.dma_start(Wdt_sb[:], io["Wdt"][:, :])
        nc.sync.dma_start(ones_sb[:], io["ones"][:, :])
        nc.sync.dma_start(ident_sb[:], io["ident"][:, :])

        # ---- GEMM A: xz_T = W_in^T @ x_T ; silu on z half ----
        with tc.tile_pool(name="gin", bufs=1) as gin, tc.tile_pool(
            name="psA", bufs=4, space="PSUM"
        ) as psA:
            W_in_sb = [
                gin.tile([P, 2 * DI], f32, tag=f"Wi{i}", name=f"Wi{i}")
                for i in range(NCH)
            ]
            xT_sb = [
                gin.tile([P, L], f32, tag=f"xT{i}", name=f"xT{i}")
                for i in range(NCH)
            ]
            for i in range(NCH):
                sl = slice(i * P, (i + 1) * P)
                nc.sync.dma_start(W_in_sb[i][:], io["W_in"][sl, :])
                nc.sync.dma_start(xT_sb[i][:], io["xT"][sl, :])

            for cb in range(2 * DI // P):  # 8 output blocks of 128 channels
                for tb in range(NTB):
                    ps = psA.tile([P, TB], f32, tag="psA", name="psA")
                    for mk in range(NCH):
                        _mm(
                            nc, ps[:],
                            W_in_sb[mk][:, cb * P : (cb + 1) * P],
                            xT_sb[mk][:, tb * TB : (tb + 1) * TB],
                            start=(mk == 0), stop=(mk == NCH - 1),
                            f32r=USE_F32R_GEMM,
                        )
                    tsl = slice(tb * TB, (tb + 1) * TB)
                    if cb < NCH:
                        nc.scalar.activation(xc_sb[cb][:, tsl], ps[:], AF.Copy)
                    else:
                        nc.scalar.activation(zs_sb[cb - NCH][:, tsl], ps[:], AF.Silu)

        # ---- causal depthwise conv (K=4) + silu -> xs ----
        with tc.tile_pool(name="cv", bufs=2) as cvp:
            for dc in range(NCH):
                xcv = cvp.tile([P, L], f32, tag="xcv", name="xcv")
                nc.vector.tensor_scalar_mul(xcv[:], xc_sb[dc][:], Wc_sb[dc][:, 3:4])
                for k in (2, 1, 0):
                    s = K - 1 - k
                    nc.vector.scalar_tensor_tensor(
                        out=xcv[:, s:],
                        in0=xc_sb[dc][:, : L - s],
                        scalar=Wc_sb[dc][:, k : k + 1],
                        in1=xcv[:, s:],
                        op0=OP.mult,
                        op1=OP.add,
                    )
                nc.scalar.activation(
                    xs_sb[dc][:], xcv[:], AF.Silu, bias=bconv_sb[dc][:, 0:1]
                )

        # ---- GEMM B: dbc_T = W_xproj^T @ xs_T  (64 rows: dt_in | B | C) ----
        with tc.tile_pool(name="psB", bufs=2, space="PSUM") as psB:
            for tb in range(NTB):
                ps = psB.tile([R + 2 * N, TB], f32, tag="psB", name="psB")
                for dc in range(NCH):
                    _mm(
                        nc, ps[:], Wx_sb[dc][:],
                        xs_sb[dc][:, tb * TB : (tb + 1) * TB],
                        start=(dc == 0), stop=(dc == NCH - 1),
                        f32r=USE_F32R_GEMM,
                    )
                nc.scalar.activation(
                    dbc_sb[:, tb * TB : (tb + 1) * TB], ps[:], AF.Copy
                )

        # ---- GEMM C: dt_T = softplus(W_dt^T @ dt_in_T + b_dt) ----
        # softplus(x) = ln(1 + exp(x)); the ACT softplus table isn't available,
        # but exp and ln share one table set (natural_log_exp_and_others).
        with tc.tile_pool(name="psC", bufs=2, space="PSUM") as psC, tc.tile_pool(
            name="spl", bufs=2
        ) as spl:
            for dc in range(NCH):
                for tb in range(NTB):
                    ps = psC.tile([P, TB], f32, tag="psC", name="psC")
                    _mm(
                        nc, ps[:], Wdt_sb[:, dc * P : (dc + 1) * P],
                        dbc_sb[0:R, tb * TB : (tb + 1) * TB],
                        start=True, stop=True, f32r=USE_F32R_GEMM,
                    )
                    et = spl.tile([P, TB], f32, tag="et", name="et")
                    nc.scalar.activation(
                        et[:], ps[:], AF.Exp, bias=bdt_sb[dc][:, 0:1]
                    )
                    nc.scalar.activation(
                        dt_sb[dc][:, tb * TB : (tb + 1) * TB],
                        et[:],
                        AF.Ln,
                        bias=1.0,
                    )

        # u = dt * xs
        for dc in range(NCH):
            nc.vector.tensor_mul(u_sb[dc][:], dt_sb[dc][:], xs_sb[dc][:])

        # ---- selective scan ----
        # Two passes over chunk pairs so the per-chunk y accumulators (PSUM)
        # and the B/C broadcast tiles fit in the 8 PSUM banks together.
        with tc.tile_pool(name="scan", bufs=3) as sp, tc.tile_pool(
            name="rowp", bufs=3
        ) as rowp, tc.tile_pool(name="cbp", bufs=2) as cbp, tc.tile_pool(
            name="psbc", bufs=2, space="PSUM"
        ) as psbc, tc.tile_pool(name="psy", bufs=1, space="PSUM") as psy:
            for half in range(2):
                chunks = (2 * half, 2 * half + 1)
                y_ps = {
                    dc: psy.tile([P, L], f32, tag=f"y{dc % 2}", name=f"y{dc % 2}")
                    for dc in chunks
                }
                for n in range(N):
                    Brow = rowp.tile([1, L], F32R, tag="Brow", name="Brow")
                    nc.sync.dma_start(Brow[:], dbc_sb[R + n : R + n + 1, :].bitcast(F32R))
                    Crow = rowp.tile([1, L], F32R, tag="Crow", name="Crow")
                    nc.sync.dma_start(Crow[:], dbc_sb[R + N + n : R + N + n + 1, :].bitcast(F32R))
                    Bb = psbc.tile([P, L], f32, tag="bc", name="Bb")
                    Cb_ps = psbc.tile([P, L], f32, tag="bc", name="Cb_ps")
                    for tb in range(NTB):
                        tsl = slice(tb * TB, (tb + 1) * TB)
                        _mm(nc, Bb[:, tsl], ones_sb[:], Brow[:, tsl],
                            start=True, stop=True, f32r=USE_F32R_AUX)
                        _mm(nc, Cb_ps[:, tsl], ones_sb[:], Crow[:, tsl],
                            start=True, stop=True, f32r=USE_F32R_AUX)
                    Cb = cbp.tile([P, L], f32, tag="Cb", name="Cb")
                    nc.scalar.activation(Cb[:], Cb_ps[:], AF.Copy)

                    for dc in chunks:
                        dA = sp.tile([P, L], f32, tag="dA", name="dA")
                        nc.scalar.activation(
                            dA[:], dt_sb[dc][:], AF.Exp,
                            scale=A_sb[dc][:, n : n + 1],
                        )
                        dBx = sp.tile([P, L], f32, tag="dBx", name="dBx")
                        nc.vector.tensor_mul(dBx[:], u_sb[dc][:], Bb[:])
                        h = sp.tile([P, L], f32, tag="h", name="h")
                        nc.vector.tensor_tensor_scan(
                            h[:], dA[:], dBx[:], 0.0, op0=OP.mult, op1=OP.add
                        )
                        hC = sp.tile([P, L], F32R, tag="hC", name="hC")
                        nc.gpsimd.tensor_tensor(hC[:], h[:], Cb[:], op=OP.mult)
                        # y += hC via identity matmul (PSUM accumulate)
                        for tb in range(NTB):
                            tsl = slice(tb * TB, (tb + 1) * TB)
                            _mm(nc, y_ps[dc][:, tsl], ident_sb[:], hC[:, tsl],
                                start=(n == 0), stop=(n == N - 1),
                                f32r=USE_F32R_AUX)

                # yz = (y + D*xs) * silu(z)
                for dc in chunks:
                    nc.vector.scalar_tensor_tensor(
                        out=yz_sb[dc][:],
                        in0=xs_sb[dc][:],
                        scalar=D_sb[dc][:, 0:1],
                        in1=y_ps[dc][:],
                        op0=OP.mult,
                        op1=OP.add,
                    )
                    nc.vector.tensor_mul(yz_sb[dc][:], yz_sb[dc][:], zs_sb[dc][:])

        # ---- GEMM D: out_T = W_out^T @ yz_T ----
        with tc.tile_pool(name="psD", bufs=4, space="PSUM") as psD, tc.tile_pool(
            name="osb", bufs=4
        ) as osb:
            for mb in range(DM // P):
                for tb in range(NTB):
                    ps = psD.tile([P, TB], f32, tag="psD", name="psD")
                    for dc in range(NCH):
                        _mm(
                            nc, ps[:],
                            Wout_sb[dc][:, mb * P : (mb + 1) * P],
                            yz_sb[dc][:, tb * TB : (tb + 1) * TB],
                            start=(dc == 0), stop=(dc == NCH - 1),
                            f32r=USE_F32R_GEMM,
                        )
                    ot = osb.tile([P, TB], f32, tag="ot", name="ot")
                    nc.scalar.activation(ot[:], ps[:], AF.Copy)
                    nc.sync.dma_start(
                        io["outT"][mb * P : (mb + 1) * P, tb * TB : (tb + 1) * TB],
                        ot[:],
                    )


def build(reps=1):
    nc = bacc.Bacc(
        "TRN2",
        target_bir_lowering=False,
        debug=False,
        enable_asserts=False,
        num_devices=N_CORES,
    )
    io = {
        "xT": nc.dram_tensor("xT", (DM, L), F32, kind="ExternalInput").ap(),
        "W_in": nc.dram_tensor("W_in", (DM, 2 * DI), F32, kind="ExternalInput").ap(),
        "Wc": nc.dram_tensor("Wc", (DI, K), F32, kind="ExternalInput").ap(),
        "bconv": nc.dram_tensor("bconv", (DI, 1), F32, kind="ExternalInput").ap(),
        "Wx": nc.dram_tensor("Wx", (DI, R + 2 * N), F32, kind="ExternalInput").ap(),
        "Wdt": nc.dram_tensor("Wdt", (R, DI), F32, kind="ExternalInput").ap(),
        "bdt": nc.dram_tensor("bdt", (DI, 1), F32, kind="ExternalInput").ap(),
        "A_sc": nc.dram_tensor("A_sc", (DI, N), F32, kind="ExternalInput").ap(),
        "Dv": nc.dram_tensor("Dv", (DI, 1), F32, kind="ExternalInput").ap(),
        "W_out": nc.dram_tensor("W_out", (DI, DM), F32, kind="ExternalInput").ap(),
        "ones": nc.dram_tensor("ones", (1, P), F32R, kind="ExternalInput").ap(),
        "ident": nc.dram_tensor("ident", (P, P), F32R, kind="ExternalInput").ap(),
        "outT": nc.dram_tensor("outT", (DM, L), F32, kind="ExternalOutput").ap(),
    }
    with tile.TileContext(nc) as tc:
        if reps == 1:
            emit_mamba(tc, io)
        else:
            with tc.For_i(0, reps, 1):
                emit_mamba(tc, io)
    nc.compile()
    return nc


_NC_CACHE = {}


def _get_nc(reps=1):
    if reps not in _NC_CACHE:
        _NC_CACHE[reps] = build(reps)
    return _NC_CACHE[reps]


def make_in_maps(inputs):
    x = np.asarray(inputs["x"], np.float32)
    in_maps = []
    for c in range(N_CORES):
        b = c % 4
        sfx = "f" if c < 4 else "b"
        xb = x[b] if c < 4 else x[b][::-1]

        def g(name):
            return np.asarray(inputs[f"{name}_{sfx}"], np.float32)

        in_maps.append(
            {
                "xT": np.ascontiguousarray(xb.T),
                "W_in": np.ascontiguousarray(g("W_in")),
                "Wc": np.ascontiguousarray(g("W_conv")),
                "bconv": np.ascontiguousarray(g("b_conv").reshape(DI, 1)),
                "Wx": np.ascontiguousarray(g("W_xproj")),
                "Wdt": np.ascontiguousarray(g("W_dt")),
                "bdt": np.ascontiguousarray(g("b_dt").reshape(DI, 1)),
                "A_sc": np.ascontiguousarray(-np.exp(g("A_log"))),
                "Dv": np.ascontiguousarray(g("D").reshape(DI, 1)),
                "W_out": np.ascontiguousarray(g("W_out")),
                "ones": np.ones((1, P), np.float32),
                "ident": np.eye(P, dtype=np.float32),
            }
        )
    return in_maps


def assemble_output(results):
    out = np.empty((4, L, DM), np.float32)
    for b in range(4):
        of = results[b]["outT"].T
        ob = results[4 + b]["outT"].T[::-1]
        out[b] = of + ob
    return out


def kernel(**inputs):
    nc = _get_nc()
    in_maps = make_in_maps(inputs)
    res = run_bass_kernel_spmd(nc, in_maps, core_ids=list(range(N_CORES)))
    return assemble_output(res.results)
